# revision 21
# baseline (speedup 1.0000x reference)
"""Trainium2 Bass kernel for nn_AtomicKANLayer.

Math: y[b,o] = sum_{i,d} fupn((x[b,i]-centers[d])*compression[d]) * coeffs[i,o,d]
with fupn the atomic basis function evaluated via its (truncated) Fourier
series.  The series phases are linear in the term index k (t_k = pi*k/a), and
compression is uniform, so with theta_k = t_k*comp:

  fupn(z_d) = mask_d * (0.5 + sum_k c_k [cos(theta_k x)cos(phi_kd) +
                                          sin(theta_k x)sin(phi_kd)]) / a

i.e. a dense matmul over per-element trig features cos/sin(theta_k x).  The
Fourier coefficients c_k decay superpolynomially; NK=31 terms leave ~5e-7
truncation error (far below the fp16 operand noise), so the 124 trig rows of
TWO 2048-element row-halves pack into the 128 partitions (aux x/ones rows
live at partitions 124..127) and every elementwise pass runs at full lane
utilization on half the free size.

Device pipeline per core (data-parallel over batch, 32 rows of B=256 each):
  1. q[p,c] = theta_p/2pi * x + off_p  as an exact split-fp16 matmul:
     theta = th+tl, x = xh+xl in fp16, 9-row contraction, fp32 PSUM   [PE]
  2. f = q - round(q) (fp32 magic-constant trick: ts add/sub, tt sub) [DVE]
  3. trig[0:124] = Sin(2pi f) -> fp16 (ACT free scale)                [ACT]
  4. [s/a | z] = W_h.T @ trig-chunk (fp16 K=128; W_h zeroes the other
     half's rows) per 128-row chunk                                   [PE]
  5. basis = s/a * (z^2 <= a^2): Square + fused select, per PSUM bank [ACT+DVE]
  6. y = sum_d basis_d.T @ coeffs_d (33 fp16 matmuls, fp32 PSUM)      [PE]
All input DMAs issue in FIFO order on the scalar queue (x first) so the
small phase-critical transfers aren't stuck behind the 2.2MB coefficient
stream; coefficients arrive in 4 d-chunks consumed in order by step 6.
"""
import sys

sys.path.insert(0, "/opt/trn_rl_repo")

import numpy as np

BF16 = np.float16  # half precision for PE operands (fp32 PSUM accumulate)
B, I, O, D = 256, 128, 256, 33
NCORES = 8
BLOC = B // NCORES          # 32 batch rows per core
ROWS = BLOC * I             # 4096 flattened (b, i) elements per core
HALF = ROWS // 2            # rows per packed partition-half
NK = 31                     # Fourier terms kept (of reference's 100)
NTRIG = 2 * NK              # cos/sin rows per half
N_ORDER, NPROD = 1, 10
A_SUP = (N_ORDER + 2) / 2.0  # support half-width a = 1.5
MAGIC = float(np.float32(1.5 * 2**23))
TWO_PI = float(2 * np.pi)
PCH = 1024                  # phase-chain chunk (columns of the packed half)
NPCH = HALF // PCH          # 2
CO_DMA_CHUNKS = 4
WAVE = 7                    # s/z chunks per PSUM bank
CO_SCALE = 256.0            # lift fp16 coeffs out of subnormal range

_PROG = None


def _build_program():
    import concourse.bacc as bacc
    import concourse.tile as tile
    from concourse import mybir

    f32 = mybir.dt.float32
    f16 = mybir.dt.float16
    Alu = mybir.AluOpType
    Act = mybir.ActivationFunctionType

    nc = bacc.Bacc("TRN2", target_bir_lowering=False, debug=False,
                   num_devices=NCORES)
    uq_d = nc.dram_tensor("uq", [9, HALF], f16, kind="ExternalInput")
    xs_d = nc.dram_tensor("xs", [4, HALF], f16, kind="ExternalInput")
    pq_d = nc.dram_tensor("pq", [9, 128], f16, kind="ExternalInput")
    w_d = nc.dram_tensor("w", [128, 4 * D], f16, kind="ExternalInput")
    co_d = nc.dram_tensor("co", [I, D * O], f16, kind="ExternalInput")
    y_d = nc.dram_tensor("y_s", [BLOC, O], f32, kind="ExternalOutput")

    with tile.TileContext(nc) as tc:
        with (
            tc.tile_pool(name="const", bufs=1) as cpool,
            tc.tile_pool(name="work", bufs=2) as wpool,
            tc.tile_pool(name="qp", bufs=2, space="PSUM") as qpool,
            tc.tile_pool(name="szp", bufs=3, space="PSUM") as szpool,
            tc.tile_pool(name="yp", bufs=1, space="PSUM") as ypool,
        ):
            # all input DMAs on the sync queue (scalar is blocked by the ACT
            # table load), phase-critical first
            uq_t = cpool.tile([9, HALF], f16)
            nc.sync.dma_start(uq_t[:], uq_d.ap()[:])
            pq_t = cpool.tile([9, 128], f16)
            nc.sync.dma_start(pq_t[:], pq_d.ap()[:])
            co_t = cpool.tile([I, D * O], f16)
            dper = (D + CO_DMA_CHUNKS - 1) // CO_DMA_CHUNKS
            for c in range(CO_DMA_CHUNKS):
                d0, d1 = c * dper, min(D, (c + 1) * dper)
                nc.sync.dma_start(co_t[:, O * d0:O * d1],
                                  co_d.ap()[:, O * d0:O * d1])
            # trig features, packed: partitions [0..61] / [62..123] = cos/sin
            # of row-half 0 / 1; partitions 124..127 = [x0; 1; x1; 1] aux
            trig = cpool.tile([128, HALF], f16)
            nc.sync.dma_start(trig[NTRIG * 2:128, :], xs_d.ap()[:])
            w_t = cpool.tile([128, 4 * D], f16)
            nc.sync.dma_start(w_t[:], w_d.ap()[:])

            import contextlib
            for ch in range(NPCH):
                cs = slice(ch * PCH, (ch + 1) * PCH)
                q = qpool.tile([128, PCH], f32, tag="q")
                for half in range(PCH // 512):  # one PSUM bank per matmul
                    qs = slice(half * 512, (half + 1) * 512)
                    nc.tensor.matmul(q[:, qs], pq_t[:],
                                     uq_t[:, ch * PCH + half * 512:
                                          ch * PCH + (half + 1) * 512],
                                     start=True, stop=True)
                # chunk 0's chain at high priority: the first Sin gates the
                # first s/z matmuls, so it must not queue behind chunk 1
                prio = tc.high_priority() if ch == 0 else contextlib.nullcontext()
                with prio:
                    qr = wpool.tile([128, PCH], f32, tag="qr")
                    nc.vector.tensor_scalar(qr[:], q[:], MAGIC, MAGIC,
                                            op0=Alu.add, op1=Alu.subtract)
                    f = wpool.tile([128, PCH], f32, tag="f")
                    nc.vector.tensor_tensor(f[:], q[:], qr[:],
                                            op=Alu.subtract)
                    nc.scalar.activation(trig[0:2 * NTRIG, cs],
                                         f[0:2 * NTRIG, :], Act.Sin,
                                         scale=TWO_PI)

            # s/z matmuls per 128-row (= one b) chunk; 7 chunks per PSUM bank
            # (66-col outputs must not cross the 512-f32 bank boundary).
            # bas is b-major (free = b*D + d): contiguous mask writes,
            # strided lhsT in the final matmuls.
            bas = cpool.tile([I, BLOC * D], f16)
            nwaves = (BLOC + WAVE - 1) // WAVE
            for wv in range(nwaves):
                b0 = wv * WAVE
                nb = min(WAVE, BLOC - b0)
                sz = szpool.tile([I, 512], f32, tag="sz")
                for j in range(nb):
                    b = b0 + j
                    h, bl = divmod(b, BLOC // 2)
                    nc.tensor.matmul(sz[:, 66 * j:66 * j + 66],
                                     trig[:, I * bl:I * bl + I],
                                     w_t[:, 66 * h:66 * h + 66],
                                     start=True, stop=True)
                szv = sz[:, 0:66 * nb].rearrange("p (j c) -> p j c", c=66)
                s_v = szv[:, :, 0:D]
                z_v = szv[:, :, D:2 * D]
                # basis = s * (z^2 <= a^2)
                m = wpool.tile([I, WAVE * D], f32, tag="m")
                m_v = m[:, 0:nb * D]
                nc.scalar.activation(
                    m_v.rearrange("p (j c) -> p j c", c=D), z_v, Act.Square)
                nc.vector.scalar_tensor_tensor(
                    bas[:, D * b0:D * (b0 + nb)].rearrange(
                        "p (j c) -> p j c", c=D),
                    in0=m_v.rearrange("p (j c) -> p j c", c=D),
                    scalar=A_SUP * A_SUP, in1=s_v,
                    op0=Alu.is_le, op1=Alu.mult)

            # final contraction: accumulate over d (lhsT strided over b-major
            # bas: column b at free index b*D + d)
            basb = bas[:].rearrange("p (b c) -> p c b", c=D)
            y_t = ypool.tile([BLOC, O], f32)
            for d in range(D):
                nc.tensor.matmul(y_t[:], basb[:, d, :],
                                 co_t[:, O * d:O * (d + 1)],
                                 start=(d == 0), stop=(d == D - 1))
            y_s = cpool.tile([BLOC, O], f32)
            nc.vector.tensor_scalar(y_s[:], y_t[:], 1.0 / CO_SCALE, None,
                                    op0=Alu.mult)
            nc.sync.dma_start(y_d.ap()[:], y_s[:])

    nc.compile()
    return nc


def _host_constants(compression, centers):
    comp = np.asarray(compression, np.float64)
    cent = np.asarray(centers, np.float64)
    assert comp.shape == (D,) and cent.shape == (D,)
    assert np.all(comp == comp[0]), "kernel assumes uniform compression"
    cval = comp[0]

    k = np.arange(1, NK + 1, dtype=np.float64)
    t = (np.pi / A_SUP) * k
    sinc = lambda z: np.sinc(z / np.pi)
    c = sinc(t / 2.0) ** N_ORDER
    for j in range(1, NPROD + 1):
        c = c * sinc(t / (2.0 ** j))

    # per-partition phase constants: theta/2pi split into fp16 hi+lo, and
    # the 1/4-turn offset for cos rows.  Partition map: p in [0,62) = half-0
    # features (cos k then sin k), [62,124) = half-1 features, 124.. aux.
    th = np.zeros(128, np.float64)
    off = np.zeros(128, np.float64)
    feat = np.concatenate([t * cval / (2 * np.pi)] * 2)   # cos then sin
    foff = np.concatenate([np.full(NK, 0.25), np.zeros(NK)])
    th[0:NTRIG] = feat
    th[NTRIG:2 * NTRIG] = feat
    off[0:NTRIG] = foff
    off[NTRIG:2 * NTRIG] = foff
    th_h = th.astype(BF16).astype(np.float64)
    th_l = (th - th_h).astype(BF16).astype(np.float64)
    pq = np.zeros((9, 128), np.float64)
    pq[0, 0:NTRIG] = th_h[0:NTRIG]
    pq[1, 0:NTRIG] = th_h[0:NTRIG]
    pq[2, 0:NTRIG] = th_l[0:NTRIG]
    pq[3, 0:NTRIG] = th_l[0:NTRIG]
    pq[4, NTRIG:2 * NTRIG] = th_h[NTRIG:2 * NTRIG]
    pq[5, NTRIG:2 * NTRIG] = th_h[NTRIG:2 * NTRIG]
    pq[6, NTRIG:2 * NTRIG] = th_l[NTRIG:2 * NTRIG]
    pq[7, NTRIG:2 * NTRIG] = th_l[NTRIG:2 * NTRIG]
    pq[8, :] = off

    # feature->output weights; W_h zeroes the other half's trig rows
    phi = np.outer(t * cval, cent)              # (NK, D)
    wf = np.zeros((NTRIG, 2 * D), np.float64)
    wf[:NK, :D] = (c[:, None] * np.cos(phi)) / A_SUP
    wf[NK:, :D] = (c[:, None] * np.sin(phi)) / A_SUP
    w = np.zeros((128, 4 * D), np.float64)
    for h in range(2):
        blk = slice(2 * D * h, 2 * D * (h + 1))
        w[NTRIG * h:NTRIG * (h + 1), blk] = wf
        w[2 * NTRIG + 2 * h, 2 * D * h + D:2 * D * (h + 1)] = cval  # x row->z
        w[2 * NTRIG + 2 * h + 1, 2 * D * h:2 * D * h + D] = 0.5 / A_SUP
        w[2 * NTRIG + 2 * h + 1, 2 * D * h + D:2 * D * (h + 1)] = -cval * cent
    return pq.astype(BF16), w.astype(BF16)


def _run(inputs, trace=False, **kw):
    global _PROG
    from concourse.bass_utils import run_bass_kernel_spmd

    if _PROG is None:
        _PROG = _build_program()
    nc = _PROG

    x = np.ascontiguousarray(np.asarray(inputs["x"], np.float32))
    coeffs = np.asarray(inputs["atomic_coeffs"], np.float32)
    pq, w = _host_constants(inputs["compression"], inputs["centers"])
    co = np.ascontiguousarray(
        (coeffs.transpose(0, 2, 1) * CO_SCALE).astype(BF16).reshape(I, D * O))

    in_maps = []
    for cid in range(NCORES):
        xflat = x[cid * BLOC:(cid + 1) * BLOC].reshape(ROWS)
        uh = xflat.astype(BF16)
        ul = (xflat - uh.astype(np.float32)).astype(BF16)
        uq = np.empty((9, HALF), BF16)
        uq[0] = uq[2] = uh[:HALF]
        uq[1] = uq[3] = ul[:HALF]
        uq[4] = uq[6] = uh[HALF:]
        uq[5] = uq[7] = ul[HALF:]
        uq[8] = BF16(1.0)
        xs = np.empty((4, HALF), BF16)
        xs[0] = uh[:HALF]
        xs[1] = BF16(1.0)
        xs[2] = uh[HALF:]
        xs[3] = BF16(1.0)
        in_maps.append({"uq": uq, "xs": xs, "pq": pq, "w": w, "co": co})

    res = run_bass_kernel_spmd(nc, in_maps, core_ids=list(range(NCORES)),
                               trace=trace, **kw)
    y = np.concatenate([res.results[c]["y_s"] for c in range(NCORES)], axis=0)
    return y.astype(np.float32, copy=False), res


def kernel(**inputs):
    y, _ = _run(inputs, trace=False)
    return y


# revision 22
# speedup vs baseline: 1.1235x; 1.1235x over previous
"""Trainium2 Bass kernel for nn_AtomicKANLayer.

Math: y[b,o] = sum_{i,d} fupn((x[b,i]-centers[d])*compression[d]) * coeffs[i,o,d]
with fupn the atomic basis function evaluated via its (truncated) Fourier
series.  The series phases are linear in the term index k (t_k = pi*k/a), and
compression is uniform, so with theta_k = t_k*comp:

  fupn(z_d) = mask_d * (0.5 + sum_k c_k [cos(theta_k x)cos(phi_kd) +
                                          sin(theta_k x)sin(phi_kd)]) / a

i.e. a dense matmul over per-element trig features cos/sin(theta_k x).  The
Fourier coefficients c_k decay superpolynomially; NK=31 terms leave ~5e-7
truncation error (far below the fp16 operand noise), so the 124 trig rows of
TWO 2048-element row-halves pack into the 128 partitions (aux x/ones rows
live at partitions 124..127) and every elementwise pass runs at full lane
utilization on half the free size.

Device pipeline per core (data-parallel over batch, 32 rows of B=256 each):
  1. q[p,c] = theta_p/2pi * x + off_p  as an exact split-fp16 matmul:
     theta = th+tl, x = xh+xl in fp16, 9-row contraction, fp32 PSUM   [PE]
  2. f = q - round(q) (fp32 magic-constant trick: ts add/sub, tt sub) [DVE]
  3. trig[0:124] = Sin(2pi f) -> fp16 (ACT free scale)                [ACT]
  4. [s/a | z] = W_h.T @ trig-chunk (fp16 K=128; W_h zeroes the other
     half's rows) per 128-row chunk                                   [PE]
  5. basis = s/a * (z^2 <= a^2): Square + fused select, per PSUM bank [ACT+DVE]
  6. y = sum_d basis_d.T @ coeffs_d (33 fp16 matmuls, fp32 PSUM)      [PE]
All input DMAs issue in FIFO order on the scalar queue (x first) so the
small phase-critical transfers aren't stuck behind the 2.2MB coefficient
stream; coefficients arrive in 4 d-chunks consumed in order by step 6.
"""
import sys

sys.path.insert(0, "/opt/trn_rl_repo")

import numpy as np

BF16 = np.float16  # half precision for PE operands (fp32 PSUM accumulate)
B, I, O, D = 256, 128, 256, 33
NCORES = 8
BLOC = B // NCORES          # 32 batch rows per core
ROWS = BLOC * I             # 4096 flattened (b, i) elements per core
HALF = ROWS // 2            # rows per packed partition-half
NK = 31                     # Fourier terms kept (of reference's 100)
NTRIG = 2 * NK              # cos/sin rows per half
N_ORDER, NPROD = 1, 10
A_SUP = (N_ORDER + 2) / 2.0  # support half-width a = 1.5
MAGIC = float(np.float32(1.5 * 2**23))
TWO_PI = float(2 * np.pi)
PCH = 1024                  # phase-chain chunk (columns of the packed half)
NPCH = HALF // PCH          # 2
CO_DMA_CHUNKS = 4
WAVE = 7                    # s/z chunks per PSUM bank
CO_SCALE = 256.0            # lift fp16 coeffs out of subnormal range

_PROG = None


def _build_program():
    import concourse.bacc as bacc
    import concourse.tile as tile
    from concourse import mybir

    f32 = mybir.dt.float32
    f16 = mybir.dt.float16
    Alu = mybir.AluOpType
    Act = mybir.ActivationFunctionType

    nc = bacc.Bacc("TRN2", target_bir_lowering=False, debug=False,
                   num_devices=NCORES)
    uq_d = nc.dram_tensor("uq", [9, HALF], f16, kind="ExternalInput")
    xs_d = nc.dram_tensor("xs", [4, HALF], f16, kind="ExternalInput")
    pq_d = nc.dram_tensor("pq", [9, 128], f16, kind="ExternalInput")
    w_d = nc.dram_tensor("w", [128, 4 * D], f16, kind="ExternalInput")
    co_d = nc.dram_tensor("co", [I, D * O], f16, kind="ExternalInput")
    y_d = nc.dram_tensor("y_s", [BLOC, O], f32, kind="ExternalOutput")

    with tile.TileContext(nc) as tc:
        with (
            tc.tile_pool(name="const", bufs=1) as cpool,
            tc.tile_pool(name="work", bufs=2) as wpool,
            tc.tile_pool(name="qp", bufs=2, space="PSUM") as qpool,
            tc.tile_pool(name="szp", bufs=3, space="PSUM") as szpool,
            tc.tile_pool(name="yp", bufs=1, space="PSUM") as ypool,
        ):
            # all input DMAs on the sync queue (scalar is blocked by the ACT
            # table load), phase-critical first
            uq_t = cpool.tile([9, HALF], f16)
            nc.sync.dma_start(uq_t[:], uq_d.ap()[:])
            pq_t = cpool.tile([9, 128], f16)
            nc.sync.dma_start(pq_t[:], pq_d.ap()[:])
            # trig features, packed: partitions [0..61] / [62..123] = cos/sin
            # of row-half 0 / 1; partitions 124..127 = [x0; 1; x1; 1] aux
            trig = cpool.tile([128, HALF], f16)
            nc.sync.dma_start(trig[NTRIG * 2:128, :], xs_d.ap()[:])
            w_t = cpool.tile([128, 4 * D], f16)
            nc.sync.dma_start(w_t[:], w_d.ap()[:])
            co_t = cpool.tile([I, D * O], f16)
            dper = (D + CO_DMA_CHUNKS - 1) // CO_DMA_CHUNKS
            for c in range(CO_DMA_CHUNKS):
                d0, d1 = c * dper, min(D, (c + 1) * dper)
                nc.sync.dma_start(co_t[:, O * d0:O * d1],
                                  co_d.ap()[:, O * d0:O * d1])

            from concourse.tile_rust import add_dep_helper
            import contextlib
            prev_f = None
            for ch in range(NPCH):
                cs = slice(ch * PCH, (ch + 1) * PCH)
                q = qpool.tile([128, PCH], f32, tag="q")
                for half in range(PCH // 512):  # one PSUM bank per matmul
                    qs = slice(half * 512, (half + 1) * 512)
                    nc.tensor.matmul(q[:, qs], pq_t[:],
                                     uq_t[:, ch * PCH + half * 512:
                                          ch * PCH + (half + 1) * 512],
                                     start=True, stop=True)
                # chunk 0's chain at high priority: the first Sin gates the
                # first s/z matmuls, so it must not queue behind chunk 1
                prio = tc.high_priority() if ch == 0 else contextlib.nullcontext()
                with prio:
                    qr = wpool.tile([128, PCH], f32, tag="qr")
                    qr_i = nc.vector.tensor_scalar(qr[:], q[:], MAGIC, MAGIC,
                                                   op0=Alu.add,
                                                   op1=Alu.subtract)
                    if prev_f is not None:
                        add_dep_helper(qr_i.ins, prev_f.ins, sync=False,
                                       reason="keep DVE in chunk order")
                    f = wpool.tile([128, PCH], f32, tag="f")
                    prev_f = nc.vector.tensor_tensor(f[:], q[:], qr[:],
                                                     op=Alu.subtract)
                    nc.scalar.activation(trig[0:2 * NTRIG, cs],
                                         f[0:2 * NTRIG, :], Act.Sin,
                                         scale=TWO_PI)

            # s/z matmuls per 128-row (= one b) chunk; 7 chunks per PSUM bank
            # (66-col outputs must not cross the 512-f32 bank boundary).
            # bas is b-major (free = b*D + d): contiguous mask writes,
            # strided lhsT in the final matmuls.
            bas = cpool.tile([I, BLOC * D], f16)
            nwaves = (BLOC + WAVE - 1) // WAVE
            for wv in range(nwaves):
                b0 = wv * WAVE
                nb = min(WAVE, BLOC - b0)
                sz = szpool.tile([I, 512], f32, tag="sz")
                for j in range(nb):
                    b = b0 + j
                    h, bl = divmod(b, BLOC // 2)
                    nc.tensor.matmul(sz[:, 66 * j:66 * j + 66],
                                     trig[:, I * bl:I * bl + I],
                                     w_t[:, 66 * h:66 * h + 66],
                                     start=True, stop=True)
                szv = sz[:, 0:66 * nb].rearrange("p (j c) -> p j c", c=66)
                s_v = szv[:, :, 0:D]
                z_v = szv[:, :, D:2 * D]
                # basis = s * (z^2 <= a^2)
                m = wpool.tile([I, WAVE * D], f32, tag="m")
                m_v = m[:, 0:nb * D]
                nc.scalar.activation(
                    m_v.rearrange("p (j c) -> p j c", c=D), z_v, Act.Square)
                nc.vector.scalar_tensor_tensor(
                    bas[:, D * b0:D * (b0 + nb)].rearrange(
                        "p (j c) -> p j c", c=D),
                    in0=m_v.rearrange("p (j c) -> p j c", c=D),
                    scalar=A_SUP * A_SUP, in1=s_v,
                    op0=Alu.is_le, op1=Alu.mult)

            # final contraction: accumulate over d (lhsT strided over b-major
            # bas: column b at free index b*D + d)
            basb = bas[:].rearrange("p (b c) -> p c b", c=D)
            y_t = ypool.tile([BLOC, O], f32)
            for d in range(D):
                nc.tensor.matmul(y_t[:], basb[:, d, :],
                                 co_t[:, O * d:O * (d + 1)],
                                 start=(d == 0), stop=(d == D - 1))
            y_s = cpool.tile([BLOC, O], f32)
            nc.vector.tensor_scalar(y_s[:], y_t[:], 1.0 / CO_SCALE, None,
                                    op0=Alu.mult)
            nc.sync.dma_start(y_d.ap()[:], y_s[:])

    nc.compile()
    return nc


def _host_constants(compression, centers):
    comp = np.asarray(compression, np.float64)
    cent = np.asarray(centers, np.float64)
    assert comp.shape == (D,) and cent.shape == (D,)
    assert np.all(comp == comp[0]), "kernel assumes uniform compression"
    cval = comp[0]

    k = np.arange(1, NK + 1, dtype=np.float64)
    t = (np.pi / A_SUP) * k
    sinc = lambda z: np.sinc(z / np.pi)
    c = sinc(t / 2.0) ** N_ORDER
    for j in range(1, NPROD + 1):
        c = c * sinc(t / (2.0 ** j))

    # per-partition phase constants: theta/2pi split into fp16 hi+lo, and
    # the 1/4-turn offset for cos rows.  Partition map: p in [0,62) = half-0
    # features (cos k then sin k), [62,124) = half-1 features, 124.. aux.
    th = np.zeros(128, np.float64)
    off = np.zeros(128, np.float64)
    feat = np.concatenate([t * cval / (2 * np.pi)] * 2)   # cos then sin
    foff = np.concatenate([np.full(NK, 0.25), np.zeros(NK)])
    th[0:NTRIG] = feat
    th[NTRIG:2 * NTRIG] = feat
    off[0:NTRIG] = foff
    off[NTRIG:2 * NTRIG] = foff
    th_h = th.astype(BF16).astype(np.float64)
    th_l = (th - th_h).astype(BF16).astype(np.float64)
    pq = np.zeros((9, 128), np.float64)
    pq[0, 0:NTRIG] = th_h[0:NTRIG]
    pq[1, 0:NTRIG] = th_h[0:NTRIG]
    pq[2, 0:NTRIG] = th_l[0:NTRIG]
    pq[3, 0:NTRIG] = th_l[0:NTRIG]
    pq[4, NTRIG:2 * NTRIG] = th_h[NTRIG:2 * NTRIG]
    pq[5, NTRIG:2 * NTRIG] = th_h[NTRIG:2 * NTRIG]
    pq[6, NTRIG:2 * NTRIG] = th_l[NTRIG:2 * NTRIG]
    pq[7, NTRIG:2 * NTRIG] = th_l[NTRIG:2 * NTRIG]
    pq[8, :] = off

    # feature->output weights; W_h zeroes the other half's trig rows
    phi = np.outer(t * cval, cent)              # (NK, D)
    wf = np.zeros((NTRIG, 2 * D), np.float64)
    wf[:NK, :D] = (c[:, None] * np.cos(phi)) / A_SUP
    wf[NK:, :D] = (c[:, None] * np.sin(phi)) / A_SUP
    w = np.zeros((128, 4 * D), np.float64)
    for h in range(2):
        blk = slice(2 * D * h, 2 * D * (h + 1))
        w[NTRIG * h:NTRIG * (h + 1), blk] = wf
        w[2 * NTRIG + 2 * h, 2 * D * h + D:2 * D * (h + 1)] = cval  # x row->z
        w[2 * NTRIG + 2 * h + 1, 2 * D * h:2 * D * h + D] = 0.5 / A_SUP
        w[2 * NTRIG + 2 * h + 1, 2 * D * h + D:2 * D * (h + 1)] = -cval * cent
    return pq.astype(BF16), w.astype(BF16)


def _run(inputs, trace=False, **kw):
    global _PROG
    from concourse.bass_utils import run_bass_kernel_spmd

    if _PROG is None:
        _PROG = _build_program()
    nc = _PROG

    x = np.ascontiguousarray(np.asarray(inputs["x"], np.float32))
    coeffs = np.asarray(inputs["atomic_coeffs"], np.float32)
    pq, w = _host_constants(inputs["compression"], inputs["centers"])
    co = np.ascontiguousarray(
        (coeffs.transpose(0, 2, 1) * CO_SCALE).astype(BF16).reshape(I, D * O))

    in_maps = []
    for cid in range(NCORES):
        xflat = x[cid * BLOC:(cid + 1) * BLOC].reshape(ROWS)
        uh = xflat.astype(BF16)
        ul = (xflat - uh.astype(np.float32)).astype(BF16)
        uq = np.empty((9, HALF), BF16)
        uq[0] = uq[2] = uh[:HALF]
        uq[1] = uq[3] = ul[:HALF]
        uq[4] = uq[6] = uh[HALF:]
        uq[5] = uq[7] = ul[HALF:]
        uq[8] = BF16(1.0)
        xs = np.empty((4, HALF), BF16)
        xs[0] = uh[:HALF]
        xs[1] = BF16(1.0)
        xs[2] = uh[HALF:]
        xs[3] = BF16(1.0)
        in_maps.append({"uq": uq, "xs": xs, "pq": pq, "w": w, "co": co})

    res = run_bass_kernel_spmd(nc, in_maps, core_ids=list(range(NCORES)),
                               trace=trace, **kw)
    y = np.concatenate([res.results[c]["y_s"] for c in range(NCORES)], axis=0)
    return y.astype(np.float32, copy=False), res


def kernel(**inputs):
    y, _ = _run(inputs, trace=False)
    return y
